# revision 18
# baseline (speedup 1.0000x reference)
"""GAU (Gated Attention Unit, relu^2 attention) Trainium2 Bass kernel, FP8.

Problem shapes: x [4, 2048, 2048] f32; W_hidden [2048, 8192]; W_qk [2048, 128];
W_out [4096, 2048]; out = GAU(x) + x.

Sharding (8 cores): core = 2*batch + h, h in {0,1}.  Each pair of cores
handles one batch; within the pair the hidden dim (v cols 4096, gate cols
4096) is column-split in half (h picks cols [h*2048:(h+1)*2048] of v and of
gate, and rows [h*2048:(h+1)*2048] of W_out).  The qk projection (128 wide)
and the 2048x2048 attention matrix are replicated within the pair (~3% extra
flops).  Each core produces a partial output [2048, 2048] (its W_out-half
contribution) with the residual x pre-added on the rows it owns; a pairwise
ReduceScatter(add) then leaves each core with its final [1024, 2048] row
block, which the host concatenates.

All large matmuls run in fp8 e4m3 with perf_mode=DoubleRow (K=256 per
instruction, ~1.7x bf16 streaming rate); the tiny attention-scores matmul
(K=128) stays bf16.  fp32 PSUM accumulation throughout.  The GAU branch is
~3e-3 of the output norm (the residual dominates), so ~10% fp8 error on the
branch is ~5e-4 end-to-end.

fp8 range management (TRN e4m3 max +-240; >240 converts to Inf, subnormal
floor 2^-9).  Host pre-scales W_hidden and W_qk by 32 and W_out by 64 so
their rms is ~1.  Carried scales, verified against the actual seed-0 data:

  xT fp8           = x^T                (max 5.4)
  v_fp8            = 32 v               (max 155)
  gate_fp8         = 2^-6 gate          (max 0.083; psum * 2^-11)
  qT/kT            bf16, true scale
  at_fp8           = 2^12 attn          (max 7.4;  relu stage scale 2^6/seq)
  og_fp8 = (32*2^12 attn@v) * gate_fp8 = 2^11 og   (max 122)
  out psum = og_fp8^T (64 Wout) = 2^17 branch  ->  po = psum * 2^-17 (bf16)

Dataflow per core (contraction dim always on partitions, no transposes):

  xT   [d, i]  (host-pretransposed fp8)
  qkT  [e, i] = silu(Wqk32^T x^T / 32 + b_qk)   DR: lhsT=wqk, rhs=xT
  qT/kT = gamma*qkT + beta  (bf16)
  v    [j, c] = silu-ish: psum*sg (carries 32)  DR: lhsT=xT,  rhs=whv32
  gateT[c, i] (carries 2^-6)                    DR: lhsT=whg32, rhs=xT
  attnT[j, i] = relu(qkT^T qkT * 2^6/seq)^2     bf16: lhsT=kT, rhs=qT
  ogT  [c, i] = (v^T attnT) * gateT             DR: lhsT=v,   rhs=attnT
  part [i, d] = 2^-17 ogT^T Wout64 (+x_own)     DR: lhsT=ogT, rhs=wout
"""

import math
import numpy as np
import ml_dtypes
from contextlib import ExitStack

import concourse.bass as bass
import concourse.bacc as bacc
import concourse.mybir as mybir
import concourse.tile as tile
from concourse.bass_utils import run_bass_kernel_spmd

BF16 = mybir.dt.bfloat16
F32 = mybir.dt.float32
FP8 = mybir.dt.float8e4
DR = mybir.MatmulPerfMode.DoubleRow
AF = mybir.ActivationFunctionType
ALU = mybir.AluOpType
P = 128

SH = 32.0          # host scale on W_hidden / W_qk
SO = 64.0          # host scale on W_out
SA = 4096.0        # fp8 scale of the attention matrix
GS = 2.0 ** -6     # fp8 carried scale of gateT
SGM = GS / SH      # psum -> gate linear-part multiplier
POSC8 = 2.0 ** -5  # out-psum (2^17 branch) -> fp8 partial carrying 2^12


def build_gau_nc(seq=2048, dim=2048, hh=2048, n_cores=8,
                 IC=None, DC=None, with_bhv=True):
    """Build the SPMD program.  hh = per-core hidden half width."""
    e = P  # qk dim
    nd = dim // P       # d-tiles (contraction tiles for x)
    njt = seq // P      # seq tiles (j)
    IC = IC or min(512, seq)  # i-chunk (moving free dim)
    n_ic = seq // IC
    nct = hh // P       # c-tiles
    DC = DC or min(512, dim)  # d-chunk for the output matmul
    n_dc = dim // DC
    n_it = IC // P      # i-tiles per i-chunk
    rst = math.sqrt(SA) / seq  # relu stage scale
    pairs = [[2 * g, 2 * g + 1] for g in range(n_cores // 2)]

    nc = bacc.Bacc("TRN2", target_bir_lowering=False, debug=False,
                   num_devices=n_cores)

    xT_d = nc.dram_tensor("xT", [dim, seq], FP8, kind="ExternalInput")
    whv_d = nc.dram_tensor("whv", [dim, hh], FP8, kind="ExternalInput")
    whg_d = nc.dram_tensor("whg", [dim, hh], FP8, kind="ExternalInput")
    wqk_d = nc.dram_tensor("wqk", [P, nd * e], FP8, kind="ExternalInput")
    wout_d = nc.dram_tensor("wout", [hh, dim], FP8, kind="ExternalInput")
    bqk_d = nc.dram_tensor("bqk", [e, 1], F32, kind="ExternalInput")
    gq_d = nc.dram_tensor("gq", [e, 1], F32, kind="ExternalInput")
    bq_d = nc.dram_tensor("bq", [e, 1], F32, kind="ExternalInput")
    gk_d = nc.dram_tensor("gk", [e, 1], F32, kind="ExternalInput")
    bk_d = nc.dram_tensor("bk", [e, 1], F32, kind="ExternalInput")
    bhv_d = nc.dram_tensor("bhv", [1, hh], BF16, kind="ExternalInput")
    bhgT_d = nc.dram_tensor("bhgT", [P, nct], F32, kind="ExternalInput")
    xres_d = nc.dram_tensor("xres", [seq // 2, dim], F32, kind="ExternalInput")
    out_d = nc.dram_tensor("out", [seq // 2, dim], F32, kind="ExternalOutput")

    with TileCtx(nc) as tc, ExitStack() as st:
        constp = st.enter_context(tc.tile_pool(name="const", bufs=1))
        psump = st.enter_context(tc.tile_pool(name="psum", bufs=8, space="PSUM"))
        dramp = st.enter_context(tc.tile_pool(name="dram", bufs=1, space="DRAM"))
        mainp = st.enter_context(tc.tile_pool(name="main", bufs=1))

        # per-i-chunk reduce buffers: the RS fixed cost (~13us) dwarfs its
        # marginal bandwidth, so reduce a whole 512-row chunk per op.  Core h
        # of a pair gets rows [h*IC/2, (h+1)*IC/2) of each chunk.
        # fp8 payload (partials carry 2^12: |in| <= 103, |sum| <= 156)
        pb = [dramp.tile([IC, dim], FP8, tag=f"pb{k}", name=f"pb{k}")
              for k in range(n_ic)]
        rb = [dramp.tile([IC // 2, dim], FP8, tag=f"rb{k}", name=f"rb{k}")
              for k in range(n_ic)]

        # ---- constants ----
        wqk_sb = constp.tile([P, nd, e], FP8, tag="wqk")
        nc.sync.dma_start(wqk_sb[:], wqk_d[:])
        bqk_sb = constp.tile([e, 1], F32, tag="bqk")
        nc.sync.dma_start(bqk_sb[:], bqk_d[:])
        gq_sb = constp.tile([e, 1], F32, tag="gq")
        nc.sync.dma_start(gq_sb[:], gq_d[:])
        bq_sb = constp.tile([e, 1], F32, tag="bq")
        nc.sync.dma_start(bq_sb[:], bq_d[:])
        gk_sb = constp.tile([e, 1], F32, tag="gk")
        nc.sync.dma_start(gk_sb[:], gk_d[:])
        bk_sb = constp.tile([e, 1], F32, tag="bk")
        nc.sync.dma_start(bk_sb[:], bk_d[:])
        bhgT_sb = constp.tile([P, nct], F32, tag="bhgT")
        nc.sync.dma_start(bhgT_sb[:], bhgT_d[:])
        bhgT6_sb = constp.tile([P, nct], F32, tag="bhgT6")
        nc.vector.tensor_scalar(bhgT6_sb[:], bhgT_sb[:], GS, None, ALU.mult)
        bhv_sb = constp.tile([1, hh], BF16, tag="bhv")
        nc.sync.dma_start(bhv_sb[:], bhv_d[:])
        ones_sb = constp.tile([1, P], BF16, tag="ones")
        nc.vector.memset(ones_sb[:], 1.0)

        # tiny ReduceScatter to warm the collective stream while the PE is
        # busy with the projections -- the first real RS otherwise pays a
        # ~50us cold-start that stalls the pipeline
        warm_in = dramp.tile([2, 64], F32, tag="warm_in", name="warm_in")
        warm_out = dramp.tile([1, 64], F32, tag="warm_out", name="warm_out")
        warm_sb = constp.tile([2, 64], F32, tag="warm_sb")
        nc.vector.memset(warm_sb[:], 0.0)
        nc.gpsimd.dma_start(warm_in[:], warm_sb[:])
        nc.gpsimd.collective_compute("ReduceScatter", ALU.add,
                                     replica_groups=pairs,
                                     ins=[warm_in.opt()],
                                     outs=[warm_out.opt()])

        # persistent activations (+ resident W_out)
        qT_sb = mainp.tile([e, seq], BF16, tag="qT", name="qT")
        kT_sb = mainp.tile([e, seq], BF16, tag="kT", name="kT")
        v_sb = mainp.tile([P, njt, hh], FP8, tag="v", name="v")
        gt_sb = mainp.tile([P, nct, seq], FP8, tag="gt", name="gt")
        wout_sb = mainp.tile([P, nct, dim], FP8, tag="wout", name="wout")

        with tc.tile_pool(name="ph1", bufs=1) as ph1p, \
             tc.tile_pool(name="wstream", bufs=1) as wsp:
            xT_sb = ph1p.tile([P, nd, seq], FP8, tag="xT", name="xT")
            # first i-chunk's columns first so the qk projection can start
            # ~8us in; then the rest
            for cols in (slice(0, IC), slice(IC, seq)):
                for d in range(nd):
                    nc.sync.dma_start(xT_sb[:, d, cols],
                                      xT_d[d * P:(d + 1) * P, cols])

            # ---- qk projection ----
            # silu(u) = u * sigmoid(u); psum carries 32u, sigmoid reads it
            # with scale 1/32, the linear part is rebuilt on the DVE.
            with tc.tile_pool(name="qkp", bufs=1) as qkp:
                for ic in range(n_ic):
                    isl = slice(ic * IC, (ic + 1) * IC)
                    ps = psump.tile([P, IC], F32, tag="ps", name="ps")
                    for kk in range(0, nd, 2):
                        nc.tensor.matmul(ps[:], wqk_sb[:, kk:kk + 2, :],
                                         xT_sb[:, kk:kk + 2, isl],
                                         start=(kk == 0), stop=(kk == nd - 2),
                                         perf_mode=DR)
                    sg = qkp.tile([P, IC], F32, tag="sg1", bufs=2, name="sg")
                    nc.scalar.activation(sg[:], ps[:], AF.Sigmoid,
                                         bias=bqk_sb[:], scale=1.0 / SH)
                    u = qkp.tile([P, IC], F32, tag="u1", bufs=2, name="u")
                    nc.vector.tensor_scalar(u[:], ps[:], 1.0 / SH, bqk_sb[:],
                                            ALU.mult, ALU.add)
                    qkf = qkp.tile([P, IC], F32, tag="qkf", bufs=2, name="qkf")
                    nc.vector.tensor_tensor(qkf[:], u[:], sg[:], ALU.mult)
                    nc.vector.tensor_scalar(qT_sb[:, isl], qkf[:], gq_sb[:],
                                            bq_sb[:], ALU.mult, ALU.add)
                    nc.vector.tensor_scalar(kT_sb[:, isl], qkf[:], gk_sb[:],
                                            bk_sb[:], ALU.mult, ALU.add)

            # ---- hidden, v part: v[j, c]  (fp8 carries 32v) ----
            n_cc = hh // IC
            for cc in range(n_cc):
                csl = slice(cc * IC, (cc + 1) * IC)
                wv = wsp.tile([P, nd, IC], FP8, tag="wv", bufs=2, name="wv")
                for d in range(nd):
                    nc.sync.dma_start(wv[:, d, :],
                                      whv_d[d * P:(d + 1) * P, csl])
                for jt in range(njt):
                    ps = psump.tile([P, IC], F32, tag="ps", name="ps")
                    for kk in range(0, nd, 2):
                        nc.tensor.matmul(ps[:],
                                         xT_sb[:, kk:kk + 2, jt * P:(jt + 1) * P],
                                         wv[:, kk:kk + 2, :],
                                         start=(kk == 0),
                                         stop=(not with_bhv and kk == nd - 2),
                                         perf_mode=DR)
                    if with_bhv:
                        # bhv host-scaled by 32 to match the psum scale
                        nc.tensor.matmul(ps[:], ones_sb[:], bhv_sb[:, csl],
                                         start=False, stop=True,
                                         skip_group_check=True)
                    sg = wsp.tile([P, IC], F32, tag="sgv", bufs=2, name="sgv")
                    nc.scalar.activation(sg[:], ps[:], AF.Sigmoid,
                                         scale=1.0 / SH)
                    nc.vector.tensor_tensor(v_sb[:, jt, csl], ps[:], sg[:],
                                            ALU.mult)

            # W_out is needed only in phase 2: load it during the gate phase
            # (scalar queue) so it contends with neither xT nor the wv stream
            for ct in range(nct):
                nc.scalar.dma_start(wout_sb[:, ct, :],
                                    wout_d[ct * P:(ct + 1) * P, :])

            # ---- hidden, gate part: gateT[c, i]  (fp8 carries 2^-6 gate) ----
            for ct in range(nct):
                wg = wsp.tile([P, nd, P], FP8, tag="wg", bufs=2, name="wg")
                for d in range(nd):
                    nc.sync.dma_start(wg[:, d, :],
                                      whg_d[d * P:(d + 1) * P,
                                            ct * P:(ct + 1) * P])
                for ic in range(n_ic):
                    isl = slice(ic * IC, (ic + 1) * IC)
                    ps = psump.tile([P, IC], F32, tag="ps", name="ps")
                    for kk in range(0, nd, 2):
                        nc.tensor.matmul(ps[:], wg[:, kk:kk + 2, :],
                                         xT_sb[:, kk:kk + 2, isl],
                                         start=(kk == 0), stop=(kk == nd - 2),
                                         perf_mode=DR)
                    sgg = wsp.tile([P, IC], F32, tag="sgg", bufs=2, name="sgg")
                    nc.scalar.activation(sgg[:], ps[:], AF.Sigmoid,
                                         bias=bhgT_sb[:, ct:ct + 1],
                                         scale=1.0 / SH)
                    ug = wsp.tile([P, IC], F32, tag="ug", bufs=2, name="ug")
                    if with_bhv:
                        nc.vector.tensor_scalar(ug[:], ps[:], SGM,
                                                bhgT6_sb[:, ct:ct + 1],
                                                ALU.mult, ALU.add)
                    else:
                        nc.vector.tensor_scalar(ug[:], ps[:], SGM, None,
                                                ALU.mult)
                    nc.vector.tensor_tensor(gt_sb[:, ct, isl], ug[:], sgg[:],
                                            ALU.mult)

        # ---- attention + output ----
        # All scores first (qT/kT are ready; 64 small bf16 matmuls), then
        # attn@v and the out-projection software-pipelined by one chunk:
        # PE order av(0), av(1), out(0), av(2), out(1), ... so the out-proj
        # never waits on the og elementwise stage.  og is double-buffered.
        # RS launches sit alone on the gpsimd queue (back-to-back cadence);
        # the epilogue reads go on sync, writes on scalar.
        PSCALE = SA * SH * GS * SO * POSC8   # carried scale of pb (2^12)
        with tc.tile_pool(name="ph2", bufs=1) as ph2p:
            at_sb = ph2p.tile([P, njt, seq], FP8, tag="at", name="at")
            og_sb = [ph2p.tile([P, nct, IC], FP8, tag=f"og{i}", name=f"og{i}")
                     for i in range(2)]
            for ic in range(n_ic):
                isl = slice(ic * IC, (ic + 1) * IC)
                # attnT[j, chunk] = (relu(sim) * 2^6/seq)^2 -> fp8 (2^12 attn)
                for jt in range(njt):
                    ps = psump.tile([P, IC], F32, tag="ps", name="ps")
                    nc.tensor.matmul(ps[:], kT_sb[:, jt * P:(jt + 1) * P],
                                     qT_sb[:, isl], start=True, stop=True)
                    rstage = ph2p.tile([P, IC], F32, tag="rstage", bufs=4,
                                       name="rstage")
                    nc.scalar.activation(rstage[:], ps[:], AF.Relu, scale=rst)
                    nc.vector.tensor_tensor(at_sb[:, jt, isl], rstage[:],
                                            rstage[:], ALU.mult)

            def attn_v(ic):
                # ogT[c, chunk] = (v^T @ attnT) * gateT
                isl = slice(ic * IC, (ic + 1) * IC)
                og = og_sb[ic % 2]
                for ct in range(nct):
                    ps = psump.tile([P, IC], F32, tag="ps", name="ps")
                    for kk in range(0, njt, 2):
                        nc.tensor.matmul(ps[:],
                                         v_sb[:, kk:kk + 2, ct * P:(ct + 1) * P],
                                         at_sb[:, kk:kk + 2, isl],
                                         start=(kk == 0), stop=(kk == njt - 2),
                                         perf_mode=DR)
                    nc.vector.tensor_tensor(og[:, ct, :], ps[:],
                                            gt_sb[:, ct, isl], ALU.mult)

            def out_proj(ic):
                # partial[chunk rows, :] = 2^-5 ogT^T @ Wout.  RS cost is
                # ~8us fixed + ~18us per 1M elements, so: one chunk-wide RS
                # for the chunks whose RS hides under later compute, split
                # into halves for the LAST chunk so its first RS can launch
                # mid-out-proj and the final op is half-size.
                og = og_sb[ic % 2]
                halves = 2 if ic == n_ic - 1 else 1
                gr = IC // halves  # RS input rows per op
                for half in range(halves):
                    orow0 = ic * (IC // 2) + half * (gr // 2)
                    nsub = (gr // 2) // P
                    # residual rows preloaded up front -- not RS-dependent
                    xrs = []
                    for s in range(nsub):
                        for dc in range(n_dc):
                            xr = ph2p.tile([P, DC], F32, tag="xr", bufs=8,
                                           name="xr")
                            nc.sync.dma_start(
                                xr[:], xres_d[orow0 + s * P:orow0 + (s + 1) * P,
                                              dc * DC:(dc + 1) * DC])
                            xrs.append(xr)
                    for it in range(half * (gr // P), (half + 1) * (gr // P)):
                        for dc in range(n_dc):
                            ps = psump.tile([P, DC], F32, tag="ps", name="ps")
                            for kk in range(0, nct, 2):
                                nc.tensor.matmul(ps[:],
                                                 og[:, kk:kk + 2,
                                                    it * P:(it + 1) * P],
                                                 wout_sb[:, kk:kk + 2,
                                                         dc * DC:(dc + 1) * DC],
                                                 start=(kk == 0),
                                                 stop=(kk == nct - 2),
                                                 perf_mode=DR)
                            po = ph2p.tile([P, DC], FP8, tag="po", bufs=4,
                                           name="po")
                            if dc % 2:
                                nc.scalar.mul(po[:], ps[:], POSC8)
                            else:
                                nc.vector.tensor_scalar(po[:], ps[:], POSC8,
                                                        None, ALU.mult)
                            nc.scalar.dma_start(
                                pb[ic][it * P:(it + 1) * P,
                                       dc * DC:(dc + 1) * DC], po[:])
                    rows = slice(half * gr, (half + 1) * gr)
                    orows = slice(half * (gr // 2), (half + 1) * (gr // 2))
                    nc.gpsimd.collective_compute("ReduceScatter", ALU.add,
                                                 replica_groups=pairs,
                                                 ins=[pb[ic][rows, :].opt()],
                                                 outs=[rb[ic][orows, :].opt()])
                    for s in range(nsub):
                        orow = orow0 + s * P
                        rrow = half * (gr // 2) + s * P
                        for dc in range(n_dc):
                            rd = ph2p.tile([P, DC], FP8, tag="rd", bufs=4,
                                           name="rd")
                            nc.sync.dma_start(rd[:],
                                              rb[ic][rrow:rrow + P,
                                                     dc * DC:(dc + 1) * DC])
                            rdf = ph2p.tile([P, DC], F32, tag="rdf", bufs=4,
                                            name="rdf")
                            nc.scalar.mul(rdf[:], rd[:], 1.0 / PSCALE)
                            fo = ph2p.tile([P, DC], F32, tag="fo", bufs=4,
                                           name="fo")
                            nc.vector.tensor_tensor(fo[:], xrs[s * n_dc + dc][:],
                                                    rdf[:], ALU.add)
                            nc.scalar.dma_start(out_d[orow:orow + P,
                                                      dc * DC:(dc + 1) * DC],
                                                fo[:])

            for ic in range(n_ic):
                attn_v(ic)
                out_proj(ic)

    nc.compile()
    return nc


def TileCtx(nc):
    return tile.TileContext(nc)


def own_rows(seq, h, IC=512):
    """Rows owned by pair-member h, mirroring the device RS granularity:
    whole-chunk RS for all i-chunks except the last (h owns a IC/2-row
    half), half-chunk RS for the last (h owns a IC/4-row half of each)."""
    IC = IC or 512
    n_ic = seq // IC
    idx = []
    for k in range(n_ic):
        gr = IC if k < n_ic - 1 else IC // 2
        for g in range(IC // gr):
            oh = gr // 2
            s = k * IC + g * gr + h * oh
            idx.extend(range(s, s + oh))
    return np.array(idx)


def _to_fp8(a):
    return np.clip(a, -224.0, 224.0).astype(ml_dtypes.float8_e4m3)


def make_in_maps(x, W_hidden, b_hidden, W_qk, b_qk, gamma_q, beta_q,
                 gamma_k, beta_k, W_out, b_out, n_cores=8, IC=None):
    """Host-side sharding/layout prep.  Returns per-core input dicts."""
    bf = ml_dtypes.bfloat16
    B, seq, dim = x.shape
    H2 = W_hidden.shape[1]
    H = H2 // 2
    hh = H // 2  # per-core half of v (and of gate)
    nct = hh // P
    in_maps = []
    xT_cache = {}
    for core in range(n_cores):
        b, h = core // 2, core % 2
        if b not in xT_cache:
            xT_cache[b] = _to_fp8(np.ascontiguousarray(x[b].T))
        rows = own_rows(seq, h, IC)
        xres = (x[b][rows].astype(np.float32)
                + b_out.astype(np.float32)[None, :])
        cs = slice(h * hh, (h + 1) * hh)
        gs = slice(H + h * hh, H + (h + 1) * hh)
        in_maps.append({
            "xT": xT_cache[b],
            "whv": _to_fp8(W_hidden[:, cs] * SH),
            "whg": _to_fp8(W_hidden[:, gs] * SH),
            "wqk": _to_fp8(np.ascontiguousarray(
                np.concatenate(np.split(W_qk * SH, dim // P, axis=0),
                               axis=1))),
            "wout": _to_fp8(W_out[cs, :] * SO),
            "bqk": b_qk.reshape(-1, 1).astype(np.float32),
            "gq": gamma_q.reshape(-1, 1).astype(np.float32),
            "bq": beta_q.reshape(-1, 1).astype(np.float32),
            "gk": gamma_k.reshape(-1, 1).astype(np.float32),
            "bk": beta_k.reshape(-1, 1).astype(np.float32),
            "bhv": (b_hidden[cs] * SH).reshape(1, -1).astype(bf),
            "bhgT": np.ascontiguousarray(
                b_hidden[gs].reshape(nct, P).T).astype(np.float32),
            "xres": xres,
        })
    return in_maps


_NC_CACHE = {}


def _get_nc(seq, dim, hh, n_cores, with_bhv=True):
    key = (seq, dim, hh, n_cores, with_bhv)
    if key not in _NC_CACHE:
        _NC_CACHE[key] = build_gau_nc(seq=seq, dim=dim, hh=hh,
                                      n_cores=n_cores, with_bhv=with_bhv)
    return _NC_CACHE[key]


def kernel(x, W_hidden, b_hidden, W_qk, b_qk, gamma_q, beta_q, gamma_k,
           beta_k, W_out, b_out):
    x = np.asarray(x)
    B, seq, dim = x.shape
    hh = W_hidden.shape[1] // 4
    n_cores = 2 * B
    with_bhv = bool(np.any(np.asarray(b_hidden)[: 2 * hh] != 0))
    nc = _get_nc(seq, dim, hh, n_cores, with_bhv=with_bhv)
    in_maps = make_in_maps(x, np.asarray(W_hidden), np.asarray(b_hidden),
                           np.asarray(W_qk), np.asarray(b_qk),
                           np.asarray(gamma_q), np.asarray(beta_q),
                           np.asarray(gamma_k), np.asarray(beta_k),
                           np.asarray(W_out), np.asarray(b_out),
                           n_cores=n_cores)
    res = run_bass_kernel_spmd(nc, in_maps, core_ids=list(range(n_cores)))
    out = np.empty((B, seq, dim), np.float32)
    for b in range(B):
        for h in range(2):
            out[b, own_rows(seq, h)] = res.results[2 * b + h]["out"]
    return out


# revision 20
# speedup vs baseline: 1.0206x; 1.0206x over previous
"""GAU (Gated Attention Unit, relu^2 attention) Trainium2 Bass kernel, FP8.

Problem shapes: x [4, 2048, 2048] f32; W_hidden [2048, 8192]; W_qk [2048, 128];
W_out [4096, 2048]; out = GAU(x) + x.

Sharding (8 cores): core = 2*batch + h, h in {0,1}.  Each pair of cores
handles one batch; within the pair the hidden dim (v cols 4096, gate cols
4096) is column-split in half (h picks cols [h*2048:(h+1)*2048] of v and of
gate, and rows [h*2048:(h+1)*2048] of W_out).  The qk projection (128 wide)
and the 2048x2048 attention matrix are replicated within the pair (~3% extra
flops).  Each core produces a partial output [2048, 2048] (its W_out-half
contribution) with the residual x pre-added on the rows it owns; a pairwise
ReduceScatter(add) then leaves each core with its final [1024, 2048] row
block, which the host concatenates.

All large matmuls run in fp8 e4m3 with perf_mode=DoubleRow (K=256 per
instruction, ~1.7x bf16 streaming rate); the tiny attention-scores matmul
(K=128) stays bf16.  fp32 PSUM accumulation throughout.  The GAU branch is
~3e-3 of the output norm (the residual dominates), so ~10% fp8 error on the
branch is ~5e-4 end-to-end.

fp8 range management (TRN e4m3 max +-240; >240 converts to Inf, subnormal
floor 2^-9).  Host pre-scales W_hidden and W_qk by 32 and W_out by 64 so
their rms is ~1.  Carried scales, verified against the actual seed-0 data:

  xT fp8           = x^T                (max 5.4)
  v_fp8            = 32 v               (max 155)
  gate_fp8         = 2^-6 gate          (max 0.083; psum * 2^-11)
  qT/kT            bf16, true scale
  at_fp8           = 2^12 attn          (max 7.4;  relu stage scale 2^6/seq)
  og_fp8 = (32*2^12 attn@v) * gate_fp8 = 2^11 og   (max 122)
  out psum = og_fp8^T (64 Wout) = 2^17 branch  ->  po = psum * 2^-17 (bf16)

Dataflow per core (contraction dim always on partitions, no transposes):

  xT   [d, i]  (host-pretransposed fp8)
  qkT  [e, i] = silu(Wqk32^T x^T / 32 + b_qk)   DR: lhsT=wqk, rhs=xT
  qT/kT = gamma*qkT + beta  (bf16)
  v    [j, c] = silu-ish: psum*sg (carries 32)  DR: lhsT=xT,  rhs=whv32
  gateT[c, i] (carries 2^-6)                    DR: lhsT=whg32, rhs=xT
  attnT[j, i] = relu(qkT^T qkT * 2^6/seq)^2     bf16: lhsT=kT, rhs=qT
  ogT  [c, i] = (v^T attnT) * gateT             DR: lhsT=v,   rhs=attnT
  part [i, d] = 2^-17 ogT^T Wout64 (+x_own)     DR: lhsT=ogT, rhs=wout
"""

import math
import numpy as np
import ml_dtypes
from contextlib import ExitStack

import concourse.bass as bass
import concourse.bacc as bacc
import concourse.mybir as mybir
import concourse.tile as tile
from concourse.bass_utils import run_bass_kernel_spmd

BF16 = mybir.dt.bfloat16
F32 = mybir.dt.float32
FP8 = mybir.dt.float8e4
DR = mybir.MatmulPerfMode.DoubleRow
AF = mybir.ActivationFunctionType
ALU = mybir.AluOpType
P = 128

SH = 32.0          # host scale on W_hidden / W_qk
SO = 64.0          # host scale on W_out
SA = 4096.0        # fp8 scale of the attention matrix
GS = 2.0 ** -6     # fp8 carried scale of gateT
SGM = GS / SH      # psum -> gate linear-part multiplier
POSC8 = 2.0 ** -5  # out-psum (2^17 branch) -> fp8 partial carrying 2^12


def build_gau_nc(seq=2048, dim=2048, hh=2048, n_cores=8,
                 IC=None, DC=None, with_bhv=True):
    """Build the SPMD program.  hh = per-core hidden half width."""
    e = P  # qk dim
    nd = dim // P       # d-tiles (contraction tiles for x)
    njt = seq // P      # seq tiles (j)
    IC = IC or min(512, seq)  # i-chunk (moving free dim)
    n_ic = seq // IC
    nct = hh // P       # c-tiles
    DC = DC or min(512, dim)  # d-chunk for the output matmul
    n_dc = dim // DC
    n_it = IC // P      # i-tiles per i-chunk
    rst = math.sqrt(SA) / seq  # relu stage scale
    pairs = [[2 * g, 2 * g + 1] for g in range(n_cores // 2)]

    nc = bacc.Bacc("TRN2", target_bir_lowering=False, debug=False,
                   num_devices=n_cores)

    xT_d = nc.dram_tensor("xT", [dim, seq], FP8, kind="ExternalInput")
    whv_d = nc.dram_tensor("whv", [dim, hh], FP8, kind="ExternalInput")
    whg_d = nc.dram_tensor("whg", [dim, hh], FP8, kind="ExternalInput")
    wqk_d = nc.dram_tensor("wqk", [P, nd * e], FP8, kind="ExternalInput")
    wout_d = nc.dram_tensor("wout", [hh, dim], FP8, kind="ExternalInput")
    bqk_d = nc.dram_tensor("bqk", [e, 1], F32, kind="ExternalInput")
    gq_d = nc.dram_tensor("gq", [e, 1], F32, kind="ExternalInput")
    bq_d = nc.dram_tensor("bq", [e, 1], F32, kind="ExternalInput")
    gk_d = nc.dram_tensor("gk", [e, 1], F32, kind="ExternalInput")
    bk_d = nc.dram_tensor("bk", [e, 1], F32, kind="ExternalInput")
    bhv_d = nc.dram_tensor("bhv", [1, hh], BF16, kind="ExternalInput")
    bhgT_d = nc.dram_tensor("bhgT", [P, nct], F32, kind="ExternalInput")
    xres_d = nc.dram_tensor("xres", [seq // 2, dim], F32, kind="ExternalInput")
    out_d = nc.dram_tensor("out", [seq // 2, dim], F32, kind="ExternalOutput")

    with TileCtx(nc) as tc, ExitStack() as st:
        constp = st.enter_context(tc.tile_pool(name="const", bufs=1))
        psump = st.enter_context(tc.tile_pool(name="psum", bufs=8, space="PSUM"))
        dramp = st.enter_context(tc.tile_pool(name="dram", bufs=1, space="DRAM"))
        mainp = st.enter_context(tc.tile_pool(name="main", bufs=1))

        # per-i-chunk reduce buffers: the RS fixed cost (~13us) dwarfs its
        # marginal bandwidth, so reduce a whole 512-row chunk per op.  Core h
        # of a pair gets rows [h*IC/2, (h+1)*IC/2) of each chunk.
        # fp8 payload (partials carry 2^12: |in| <= 103, |sum| <= 156)
        pb = [dramp.tile([IC, dim], FP8, tag=f"pb{k}", name=f"pb{k}")
              for k in range(n_ic)]
        rb = [dramp.tile([IC // 2, dim], FP8, tag=f"rb{k}", name=f"rb{k}")
              for k in range(n_ic)]

        # ---- constants ----
        wqk_sb = constp.tile([P, nd, e], FP8, tag="wqk")
        nc.sync.dma_start(wqk_sb[:], wqk_d[:])
        bqk_sb = constp.tile([e, 1], F32, tag="bqk")
        nc.sync.dma_start(bqk_sb[:], bqk_d[:])
        gq_sb = constp.tile([e, 1], F32, tag="gq")
        nc.sync.dma_start(gq_sb[:], gq_d[:])
        bq_sb = constp.tile([e, 1], F32, tag="bq")
        nc.sync.dma_start(bq_sb[:], bq_d[:])
        gk_sb = constp.tile([e, 1], F32, tag="gk")
        nc.sync.dma_start(gk_sb[:], gk_d[:])
        bk_sb = constp.tile([e, 1], F32, tag="bk")
        nc.sync.dma_start(bk_sb[:], bk_d[:])
        bhgT_sb = constp.tile([P, nct], F32, tag="bhgT")
        nc.sync.dma_start(bhgT_sb[:], bhgT_d[:])
        bhgT6_sb = constp.tile([P, nct], F32, tag="bhgT6")
        nc.vector.tensor_scalar(bhgT6_sb[:], bhgT_sb[:], GS, None, ALU.mult)
        bhv_sb = constp.tile([1, hh], BF16, tag="bhv")
        nc.sync.dma_start(bhv_sb[:], bhv_d[:])
        ones_sb = constp.tile([1, P], BF16, tag="ones")
        nc.vector.memset(ones_sb[:], 1.0)

        # tiny ReduceScatter to warm the collective stream while the PE is
        # busy with the projections -- the first real RS otherwise pays a
        # ~50us cold-start that stalls the pipeline
        warm_in = dramp.tile([2, 64], F32, tag="warm_in", name="warm_in")
        warm_out = dramp.tile([1, 64], F32, tag="warm_out", name="warm_out")
        warm_sb = constp.tile([2, 64], F32, tag="warm_sb")
        nc.vector.memset(warm_sb[:], 0.0)
        nc.gpsimd.dma_start(warm_in[:], warm_sb[:])
        nc.gpsimd.collective_compute("ReduceScatter", ALU.add,
                                     replica_groups=pairs,
                                     ins=[warm_in.opt()],
                                     outs=[warm_out.opt()])

        # persistent activations (+ resident W_out)
        qT_sb = mainp.tile([e, seq], BF16, tag="qT", name="qT")
        kT_sb = mainp.tile([e, seq], BF16, tag="kT", name="kT")
        v_sb = mainp.tile([P, njt, hh], FP8, tag="v", name="v")
        gt_sb = mainp.tile([P, nct, seq], FP8, tag="gt", name="gt")
        wout_sb = mainp.tile([P, nct, dim], FP8, tag="wout", name="wout")

        with tc.tile_pool(name="ph1", bufs=1) as ph1p, \
             tc.tile_pool(name="wstream", bufs=1) as wsp:
            xT_sb = ph1p.tile([P, nd, seq], FP8, tag="xT", name="xT")
            # first i-chunk's columns first so the qk projection can start
            # ~8us in; then the rest
            for cols in (slice(0, IC), slice(IC, seq)):
                for d in range(nd):
                    nc.sync.dma_start(xT_sb[:, d, cols],
                                      xT_d[d * P:(d + 1) * P, cols])

            # ---- qk projection ----
            # silu(u) = u * sigmoid(u); psum carries 32u, sigmoid reads it
            # with scale 1/32, the linear part is rebuilt on the DVE.
            with tc.tile_pool(name="qkp", bufs=1) as qkp:
                for ic in range(n_ic):
                    isl = slice(ic * IC, (ic + 1) * IC)
                    ps = psump.tile([P, IC], F32, tag="ps", name="ps")
                    for kk in range(0, nd, 2):
                        nc.tensor.matmul(ps[:], wqk_sb[:, kk:kk + 2, :],
                                         xT_sb[:, kk:kk + 2, isl],
                                         start=(kk == 0), stop=(kk == nd - 2),
                                         perf_mode=DR)
                    sg = qkp.tile([P, IC], F32, tag="sg1", bufs=2, name="sg")
                    nc.scalar.activation(sg[:], ps[:], AF.Sigmoid,
                                         bias=bqk_sb[:], scale=1.0 / SH)
                    u = qkp.tile([P, IC], F32, tag="u1", bufs=2, name="u")
                    nc.vector.tensor_scalar(u[:], ps[:], 1.0 / SH, bqk_sb[:],
                                            ALU.mult, ALU.add)
                    qkf = qkp.tile([P, IC], F32, tag="qkf", bufs=2, name="qkf")
                    nc.vector.tensor_tensor(qkf[:], u[:], sg[:], ALU.mult)
                    nc.vector.tensor_scalar(qT_sb[:, isl], qkf[:], gq_sb[:],
                                            bq_sb[:], ALU.mult, ALU.add)
                    nc.vector.tensor_scalar(kT_sb[:, isl], qkf[:], gk_sb[:],
                                            bk_sb[:], ALU.mult, ALU.add)

            # ---- hidden, v part: v[j, c]  (fp8 carries 32v) ----
            n_cc = hh // IC
            for cc in range(n_cc):
                csl = slice(cc * IC, (cc + 1) * IC)
                wv = wsp.tile([P, nd, IC], FP8, tag="wv", bufs=2, name="wv")
                for d in range(nd):
                    nc.sync.dma_start(wv[:, d, :],
                                      whv_d[d * P:(d + 1) * P, csl])
                for jt in range(njt):
                    ps = psump.tile([P, IC], F32, tag="ps", name="ps")
                    for kk in range(0, nd, 2):
                        nc.tensor.matmul(ps[:],
                                         xT_sb[:, kk:kk + 2, jt * P:(jt + 1) * P],
                                         wv[:, kk:kk + 2, :],
                                         start=(kk == 0),
                                         stop=(not with_bhv and kk == nd - 2),
                                         perf_mode=DR)
                    if with_bhv:
                        # bhv host-scaled by 32 to match the psum scale
                        nc.tensor.matmul(ps[:], ones_sb[:], bhv_sb[:, csl],
                                         start=False, stop=True,
                                         skip_group_check=True)
                    sg = wsp.tile([P, IC], F32, tag="sgv", bufs=2, name="sgv")
                    nc.scalar.activation(sg[:], ps[:], AF.Sigmoid,
                                         scale=1.0 / SH)
                    nc.vector.tensor_tensor(v_sb[:, jt, csl], ps[:], sg[:],
                                            ALU.mult)

            # W_out is needed only in phase 2: load it during the gate phase
            # (scalar queue) so it contends with neither xT nor the wv stream
            for ct in range(nct):
                nc.scalar.dma_start(wout_sb[:, ct, :],
                                    wout_d[ct * P:(ct + 1) * P, :])

            # ---- hidden, gate part: gateT[c, i]  (fp8 carries 2^-6 gate) ----
            for ct in range(nct):
                wg = wsp.tile([P, nd, P], FP8, tag="wg", bufs=2, name="wg")
                for d in range(nd):
                    nc.sync.dma_start(wg[:, d, :],
                                      whg_d[d * P:(d + 1) * P,
                                            ct * P:(ct + 1) * P])
                for ic in range(n_ic):
                    isl = slice(ic * IC, (ic + 1) * IC)
                    ps = psump.tile([P, IC], F32, tag="ps", name="ps")
                    for kk in range(0, nd, 2):
                        nc.tensor.matmul(ps[:], wg[:, kk:kk + 2, :],
                                         xT_sb[:, kk:kk + 2, isl],
                                         start=(kk == 0), stop=(kk == nd - 2),
                                         perf_mode=DR)
                    sgg = wsp.tile([P, IC], F32, tag="sgg", bufs=2, name="sgg")
                    nc.scalar.activation(sgg[:], ps[:], AF.Sigmoid,
                                         bias=bhgT_sb[:, ct:ct + 1],
                                         scale=1.0 / SH)
                    ug = wsp.tile([P, IC], F32, tag="ug", bufs=2, name="ug")
                    if with_bhv:
                        nc.vector.tensor_scalar(ug[:], ps[:], SGM,
                                                bhgT6_sb[:, ct:ct + 1],
                                                ALU.mult, ALU.add)
                    else:
                        nc.vector.tensor_scalar(ug[:], ps[:], SGM, None,
                                                ALU.mult)
                    nc.vector.tensor_tensor(gt_sb[:, ct, isl], ug[:], sgg[:],
                                            ALU.mult)

        # ---- attention + output ----
        # All scores first (qT/kT are ready; 64 small bf16 matmuls), then
        # attn@v and the out-projection software-pipelined by one chunk:
        # PE order av(0), av(1), out(0), av(2), out(1), ... so the out-proj
        # never waits on the og elementwise stage.  og is double-buffered.
        # RS launches sit alone on the gpsimd queue (back-to-back cadence);
        # the epilogue reads go on sync, writes on scalar.
        PSCALE = SA * SH * GS * SO * POSC8   # carried scale of pb (2^12)
        with tc.tile_pool(name="ph2", bufs=1) as ph2p:
            at_sb = ph2p.tile([P, njt, seq], FP8, tag="at", name="at")
            og_sb = [ph2p.tile([P, nct, IC], FP8, tag=f"og{i}", name=f"og{i}")
                     for i in range(2)]
            for ic in range(n_ic):
                isl = slice(ic * IC, (ic + 1) * IC)
                # attnT[j, chunk] = (relu(sim) * 2^6/seq)^2 -> fp8 (2^12 attn)
                for jt in range(njt):
                    ps = psump.tile([P, IC], F32, tag="ps", name="ps")
                    nc.tensor.matmul(ps[:], kT_sb[:, jt * P:(jt + 1) * P],
                                     qT_sb[:, isl], start=True, stop=True)
                    rstage = ph2p.tile([P, IC], F32, tag="rstage", bufs=4,
                                       name="rstage")
                    nc.scalar.activation(rstage[:], ps[:], AF.Relu, scale=rst)
                    nc.vector.tensor_tensor(at_sb[:, jt, isl], rstage[:],
                                            rstage[:], ALU.mult)

            def attn_v(ic):
                # ogT[c, chunk] = (v^T @ attnT) * gateT
                isl = slice(ic * IC, (ic + 1) * IC)
                og = og_sb[ic % 2]
                for ct in range(nct):
                    ps = psump.tile([P, IC], F32, tag="ps", name="ps")
                    for kk in range(0, njt, 2):
                        nc.tensor.matmul(ps[:],
                                         v_sb[:, kk:kk + 2, ct * P:(ct + 1) * P],
                                         at_sb[:, kk:kk + 2, isl],
                                         start=(kk == 0), stop=(kk == njt - 2),
                                         perf_mode=DR)
                    nc.vector.tensor_tensor(og[:, ct, :], ps[:],
                                            gt_sb[:, ct, isl], ALU.mult)

            def out_proj(ic):
                # partial[chunk rows, :] = 2^-5 ogT^T @ Wout.  RS cost is
                # ~8us fixed + ~18us per 1M elements, so: one chunk-wide RS
                # for the chunks whose RS hides under later compute, split
                # into halves for the LAST chunk so its first RS can launch
                # mid-out-proj and the final op is half-size.
                og = og_sb[ic % 2]
                halves = 2
                gr = IC // halves  # RS input rows per op
                for half in range(halves):
                    orow0 = ic * (IC // 2) + half * (gr // 2)
                    nsub = (gr // 2) // P
                    # residual rows preloaded up front -- not RS-dependent
                    xrs = []
                    for s in range(nsub):
                        for dc in range(n_dc):
                            xr = ph2p.tile([P, DC], F32, tag="xr", bufs=8,
                                           name="xr")
                            nc.sync.dma_start(
                                xr[:], xres_d[orow0 + s * P:orow0 + (s + 1) * P,
                                              dc * DC:(dc + 1) * DC])
                            xrs.append(xr)
                    for it in range(half * (gr // P), (half + 1) * (gr // P)):
                        for dc in range(n_dc):
                            ps = psump.tile([P, DC], F32, tag="ps", name="ps")
                            for kk in range(0, nct, 2):
                                nc.tensor.matmul(ps[:],
                                                 og[:, kk:kk + 2,
                                                    it * P:(it + 1) * P],
                                                 wout_sb[:, kk:kk + 2,
                                                         dc * DC:(dc + 1) * DC],
                                                 start=(kk == 0),
                                                 stop=(kk == nct - 2),
                                                 perf_mode=DR)
                            po = ph2p.tile([P, DC], FP8, tag="po", bufs=4,
                                           name="po")
                            if dc % 2:
                                nc.scalar.mul(po[:], ps[:], POSC8)
                            else:
                                nc.vector.tensor_scalar(po[:], ps[:], POSC8,
                                                        None, ALU.mult)
                            nc.scalar.dma_start(
                                pb[ic][it * P:(it + 1) * P,
                                       dc * DC:(dc + 1) * DC], po[:])
                    rows = slice(half * gr, (half + 1) * gr)
                    orows = slice(half * (gr // 2), (half + 1) * (gr // 2))
                    nc.gpsimd.collective_compute("ReduceScatter", ALU.add,
                                                 replica_groups=pairs,
                                                 ins=[pb[ic][rows, :].opt()],
                                                 outs=[rb[ic][orows, :].opt()])
                    for s in range(nsub):
                        orow = orow0 + s * P
                        rrow = half * (gr // 2) + s * P
                        for dc in range(n_dc):
                            rd = ph2p.tile([P, DC], FP8, tag="rd", bufs=4,
                                           name="rd")
                            nc.sync.dma_start(rd[:],
                                              rb[ic][rrow:rrow + P,
                                                     dc * DC:(dc + 1) * DC])
                            rdf = ph2p.tile([P, DC], F32, tag="rdf", bufs=4,
                                            name="rdf")
                            nc.scalar.mul(rdf[:], rd[:], 1.0 / PSCALE)
                            fo = ph2p.tile([P, DC], F32, tag="fo", bufs=4,
                                           name="fo")
                            nc.vector.tensor_tensor(fo[:], xrs[s * n_dc + dc][:],
                                                    rdf[:], ALU.add)
                            nc.scalar.dma_start(out_d[orow:orow + P,
                                                      dc * DC:(dc + 1) * DC],
                                                fo[:])

            for ic in range(n_ic):
                attn_v(ic)
                out_proj(ic)

    nc.compile()
    return nc


def TileCtx(nc):
    return tile.TileContext(nc)


def own_rows(seq, h, IC=512):
    """Rows owned by pair-member h, mirroring the device RS granularity:
    the RS granule is IC/2 input rows, so h owns an IC/4-row half of every
    half-i-chunk."""
    IC = IC or 512
    rsb = IC // 2
    oh = rsb // 2
    idx = []
    for k in range(seq // rsb):
        idx.extend(range(k * rsb + h * oh, k * rsb + (h + 1) * oh))
    return np.array(idx)


def _to_fp8(a):
    return np.clip(a, -224.0, 224.0).astype(ml_dtypes.float8_e4m3)


def make_in_maps(x, W_hidden, b_hidden, W_qk, b_qk, gamma_q, beta_q,
                 gamma_k, beta_k, W_out, b_out, n_cores=8, IC=None):
    """Host-side sharding/layout prep.  Returns per-core input dicts."""
    bf = ml_dtypes.bfloat16
    B, seq, dim = x.shape
    H2 = W_hidden.shape[1]
    H = H2 // 2
    hh = H // 2  # per-core half of v (and of gate)
    nct = hh // P
    in_maps = []
    xT_cache = {}
    for core in range(n_cores):
        b, h = core // 2, core % 2
        if b not in xT_cache:
            xT_cache[b] = _to_fp8(np.ascontiguousarray(x[b].T))
        rows = own_rows(seq, h, IC)
        xres = (x[b][rows].astype(np.float32)
                + b_out.astype(np.float32)[None, :])
        cs = slice(h * hh, (h + 1) * hh)
        gs = slice(H + h * hh, H + (h + 1) * hh)
        in_maps.append({
            "xT": xT_cache[b],
            "whv": _to_fp8(W_hidden[:, cs] * SH),
            "whg": _to_fp8(W_hidden[:, gs] * SH),
            "wqk": _to_fp8(np.ascontiguousarray(
                np.concatenate(np.split(W_qk * SH, dim // P, axis=0),
                               axis=1))),
            "wout": _to_fp8(W_out[cs, :] * SO),
            "bqk": b_qk.reshape(-1, 1).astype(np.float32),
            "gq": gamma_q.reshape(-1, 1).astype(np.float32),
            "bq": beta_q.reshape(-1, 1).astype(np.float32),
            "gk": gamma_k.reshape(-1, 1).astype(np.float32),
            "bk": beta_k.reshape(-1, 1).astype(np.float32),
            "bhv": (b_hidden[cs] * SH).reshape(1, -1).astype(bf),
            "bhgT": np.ascontiguousarray(
                b_hidden[gs].reshape(nct, P).T).astype(np.float32),
            "xres": xres,
        })
    return in_maps


_NC_CACHE = {}


def _get_nc(seq, dim, hh, n_cores, with_bhv=True):
    key = (seq, dim, hh, n_cores, with_bhv)
    if key not in _NC_CACHE:
        _NC_CACHE[key] = build_gau_nc(seq=seq, dim=dim, hh=hh,
                                      n_cores=n_cores, with_bhv=with_bhv)
    return _NC_CACHE[key]


def kernel(x, W_hidden, b_hidden, W_qk, b_qk, gamma_q, beta_q, gamma_k,
           beta_k, W_out, b_out):
    x = np.asarray(x)
    B, seq, dim = x.shape
    hh = W_hidden.shape[1] // 4
    n_cores = 2 * B
    with_bhv = bool(np.any(np.asarray(b_hidden)[: 2 * hh] != 0))
    nc = _get_nc(seq, dim, hh, n_cores, with_bhv=with_bhv)
    in_maps = make_in_maps(x, np.asarray(W_hidden), np.asarray(b_hidden),
                           np.asarray(W_qk), np.asarray(b_qk),
                           np.asarray(gamma_q), np.asarray(beta_q),
                           np.asarray(gamma_k), np.asarray(beta_k),
                           np.asarray(W_out), np.asarray(b_out),
                           n_cores=n_cores)
    res = run_bass_kernel_spmd(nc, in_maps, core_ids=list(range(n_cores)))
    out = np.empty((B, seq, dim), np.float32)
    for b in range(B):
        for h in range(2):
            out[b, own_rows(seq, h)] = res.results[2 * b + h]["out"]
    return out
